# revision 2
# baseline (speedup 1.0000x reference)
"""Causal multi-head attention (B=2, L=2048, D=2048, NH=16, HD=128) on 8
Trainium2 NeuronCores — mixed bf16 / fp8-DoubleRow version.

Sharding: core c = b*4 + g handles batch b and head-group g (4 heads,
F=512 features).  Host sums the 4 per-batch partial o-projections and
adds bo (+ Wo @ bv, since softmax weights sum to 1 the v-bias
contributes exactly Wo @ bv per row).

Precision scheme (errors checked against the fp32 oracle, ~9e-3 max rel
vs the 2e-2 gate):
  - tokens 0..511 ("early", query block tb=0): bf16 everywhere — the
    short-context rows can be dominated by a single v row, so v/pt
    quantization noise does not average out there.
  - tokens 512.. ("late"): fp8e4 (e4m3) with the PE's DoubleRow perf
    mode (2 contraction rows per PE cell -> 2x matmul throughput).
    Softmax rows here average >= 512 keys, so fp8 noise on v / pt / x / W
    is suppressed by the participation ratio.  pt = exp(s - 3) is used
    by BOTH the rsum and the PV matmul, so pt quantization cancels in
    the normalization ratio.  fp8 weights are pre-scaled by WS (and att
    by ASC) to stay in e4m3's normal range; descale happens in the
    PSUM->SBUF activation copy.

Structure:
  phase 1: q,k projections for the whole sequence; x loaded once and
           kept resident (bf16 early / fp8 kt-pair-interleaved late).
  phase 2: v projection from resident x (no second x stream) fused with
           flash-style causal attention and the partial o-projection,
           one flat software pipeline across (tb, head, k-subtile).
"""

import sys

for _p in ("/opt/trn_rl_repo",):
    if _p not in sys.path:
        sys.path.insert(0, _p)

import numpy as np
import ml_dtypes
from contextlib import ExitStack

import concourse.bass as bass  # noqa: F401
import concourse.tile as tile
from concourse import bacc, mybir
from concourse import bass_utils

P = 128
B, L, D = 2, 2048, 2048
NH, HD = 16, 128
SCALE = HD ** -0.5
G = 8 // B            # head-groups per batch = 4
H = 4                 # heads per core
F = H * HD            # 512 features per core
TB = 512              # token block (q-block)
NTB = L // TB         # 4
KT = D // P           # 16 contraction tiles
KTP = KT // 2         # 8 DoubleRow pair-tiles
SPLIT = 512           # attention split: q-block tb=0 stays bf16
PS = 256              # projection split: tokens below bf16, at/above fp8-DR
LT = L - PS           # 1792 fp8-projected tokens
WS = 512.0            # fp8 weight prescale
ASC = 16.0            # fp8 att prescale
ESH = -3.0            # exp shift for fp8 pt (e4m3 max 240 = e^5.48)

f32 = mybir.dt.float32
f32r = mybir.dt.float32r
bf16 = mybir.dt.bfloat16
f8e4 = mybir.dt.float8e4
DR = mybir.MatmulPerfMode.DoubleRow

_CACHE = {}


def _build(reps=1):
    key = ("nc", reps)
    if key in _CACHE:
        return _CACHE[key]

    nc = bacc.Bacc("TRN2", target_bir_lowering=False, debug=False, num_devices=8)

    xb_d = nc.dram_tensor("xb", [P, KT, PS], bf16, kind="ExternalInput").ap()
    x8_d = nc.dram_tensor("x8", [P, KTP, 2, LT], f8e4, kind="ExternalInput").ap()
    wqb_d = nc.dram_tensor("wqb", [P, H, KT, HD], bf16, kind="ExternalInput").ap()
    wq8_d = nc.dram_tensor("wq8", [P, H, KTP, 2, HD], f8e4, kind="ExternalInput").ap()
    wkb_d = nc.dram_tensor("wkb", [P, H, KT, HD], bf16, kind="ExternalInput").ap()
    wk8_d = nc.dram_tensor("wk8", [P, H, KTP, 2, HD], f8e4, kind="ExternalInput").ap()
    wvb_d = nc.dram_tensor("wvb", [P, KT, F], bf16, kind="ExternalInput").ap()
    wv8_d = nc.dram_tensor("wv8", [P, KTP, 2, F], f8e4, kind="ExternalInput").ap()
    wob_d = nc.dram_tensor("wob", [P, H, D], bf16, kind="ExternalInput").ap()
    wo8_d = nc.dram_tensor("wo8", [P, 2, 2, D], f8e4, kind="ExternalInput").ap()
    bq_d = nc.dram_tensor("bq_pp", [P, H], f32, kind="ExternalInput").ap()
    bk_d = nc.dram_tensor("bk_pp", [P, H], f32, kind="ExternalInput").ap()
    ones_bf_d = nc.dram_tensor("ones_bf", [P], bf16, kind="ExternalInput").ap()
    ones8_d = nc.dram_tensor("ones8", [P, 2, 16], f8e4, kind="ExternalInput").ap()
    ones_r_d = nc.dram_tensor("ones_r", [P], f32r, kind="ExternalInput").ap()
    o = nc.dram_tensor("o", [L, D], bf16, kind="ExternalOutput").ap()

    with tile.TileContext(nc) as tc:
        with ExitStack() as ctx:
            ctx.enter_context(nc.allow_low_precision(reason="mixed bf16/fp8 by design"))
            consts = ctx.enter_context(tc.tile_pool(name="consts", bufs=1))
            resid = ctx.enter_context(tc.tile_pool(name="resid", bufs=1))

            # ---- constants ----
            # triangular additive mask [P, P] in [k, q] orientation:
            # keep (0.0) where k_local <= q_local, else -1e30
            tri = consts.tile([P, P], f32, name="tri")
            nc.gpsimd.memset(tri[:], 0.0)
            nc.gpsimd.affine_select(
                out=tri[:],
                in_=tri[:],
                compare_op=mybir.AluOpType.is_ge,
                fill=-1e30,
                base=0,
                pattern=[[1, P]],
                channel_multiplier=-1,
            )
            m3 = consts.tile([P, 1], f32, name="m3")
            nc.gpsimd.memset(m3[:], ESH)

            ones_col = consts.tile([P, 1], bf16, name="ones_col")
            nc.scalar.dma_start(ones_col[:], ones_bf_d[:, None])
            # M=16 (duplicated columns): dual-fp8 ldweights requires M >= 16
            ones8 = consts.tile([P, 2, 16], f8e4, name="ones8")
            nc.scalar.dma_start(ones8[:], ones8_d)
            ones_row = consts.tile([1, P], f32r, name="ones_row")
            nc.scalar.dma_start(ones_row[:], ones_r_d[None, :])
            bq_pp = consts.tile([P, H], f32, name="bq_pp")
            nc.scalar.dma_start(bq_pp[:], bq_d)
            bk_pp = consts.tile([P, H], f32, name="bk_pp")
            nc.scalar.dma_start(bk_pp[:], bk_d)

            # ---- persistent activations ----
            qT_sb = resid.tile([P, H, L], bf16, name="qT_sb")     # 2 MiB
            kT_sb = resid.tile([P, H, L], bf16, name="kT_sb")     # 2 MiB
            xb_sb = resid.tile([P, KT, PS], bf16, name="xb_sb")
            x8_sb = resid.tile([P, KTP, 2, LT], f8e4, name="x8_sb")  # 3 MiB

            # phase-2 v-projection weights prefetched during phase 1
            wv_pool = ctx.enter_context(tc.tile_pool(name="wvres", bufs=1))
            wvb_res = wv_pool.tile([P, KT, F], bf16, name="wvb_res")
            wv8_res = wv_pool.tile([P, KTP, 2, F], f8e4, name="wv8_res")

            rep_ctx = ExitStack()
            if reps > 1:
                rep_ctx.enter_context(tc.For_i(0, reps, 1))

            # ================= phase 1: q,k projections =================
            with ExitStack() as p1:
                wres_pool = p1.enter_context(tc.tile_pool(name="wres", bufs=1))
                psA = p1.enter_context(tc.tile_pool(name="psA", bufs=4, space="PSUM"))

                wqb_res = wres_pool.tile([P, H, KT, HD], bf16, name="wqb_res")
                wkb_res = wres_pool.tile([P, H, KT, HD], bf16, name="wkb_res")
                wq8_res = wres_pool.tile([P, H, KTP, 2, HD], f8e4, name="wq8_res")
                wk8_res = wres_pool.tile([P, H, KTP, 2, HD], f8e4, name="wk8_res")

                # first-needed loads lead; everything else queues behind
                nc.sync.dma_start(wqb_res[:, 0], wqb_d[:, 0])
                nc.sync.dma_start(xb_sb[:], xb_d)

                # chunks: ("e", token_lo, len) bf16 / ("l", x8-local lo, len);
                # the 256-wide late chunk (tokens 256..511) runs last so its
                # x8 columns can stream in behind the 512-wide chunks
                chunks = [("e", 0, PS), ("l", PS, TB), ("l", PS + TB, TB),
                          ("l", PS + 2 * TB, TB), ("l", 0, PS)]
                for ci, (kind, lo, ln) in enumerate(chunks):
                    if ci == 0:
                        # remaining loads, in the order compute consumes them
                        # (must be emitted before chunk 0's h>=1 matmuls:
                        # program order defines def-use for Tile)
                        nc.sync.dma_start(wkb_res[:, 0], wkb_d[:, 0])
                        nc.sync.dma_start(wqb_res[:, 1:], wqb_d[:, 1:])
                        nc.sync.dma_start(wkb_res[:, 1:], wkb_d[:, 1:])
                        nc.sync.dma_start(wq8_res[:], wq8_d)
                        nc.sync.dma_start(wk8_res[:], wk8_d)
                        for llo, lln in ((PS, TB), (PS + TB, TB), (PS + 2 * TB, TB), (0, PS)):
                            nc.sync.dma_start(
                                x8_sb[:, :, :, llo : llo + lln],
                                x8_d[:, :, :, llo : llo + lln],
                            )
                        # phase-2 prefetch (DMA queue has slack during phase 1)
                        nc.sync.dma_start(wvb_res[:], wvb_d)
                        nc.sync.dma_start(wv8_res[:], wv8_d)
                    for bsel, (wb, w8, bias_pp, dst) in enumerate(
                        (
                            (wqb_res, wq8_res, bq_pp, qT_sb),
                            (wkb_res, wk8_res, bk_pp, kT_sb),
                        )
                    ):
                        for h in range(H):
                            ps = psA.tile([P, TB], f32, tag="psA")
                            if kind == "e":
                                for kt in range(KT):
                                    nc.tensor.matmul(
                                        ps[:, :ln],
                                        wb[:, h, kt],
                                        xb_sb[:, kt, lo : lo + ln],
                                        start=(kt == 0),
                                        stop=(kt == KT - 1),
                                    )
                                dlo, sc = lo, 1.0
                            else:
                                for ktp in range(KTP):
                                    nc.tensor.matmul(
                                        ps[:, :ln],
                                        w8[:, h, ktp],
                                        x8_sb[:, ktp, :, lo : lo + ln],
                                        start=(ktp == 0),
                                        stop=(ktp == KTP - 1),
                                        perf_mode=DR,
                                    )
                                dlo = PS + lo
                                sc = (SCALE if bsel == 0 else 1.0) / WS
                            nc.scalar.activation(
                                dst[:, h, dlo : dlo + ln],
                                ps[:, :ln],
                                mybir.ActivationFunctionType.Identity,
                                bias=bias_pp[:, h : h + 1],
                                scale=sc,
                            )

            # ================= phase 2: attention + o-projection ==========
            with ExitStack() as p2:
                wo_pool = p2.enter_context(tc.tile_pool(name="wop", bufs=1))
                apool = p2.enter_context(tc.tile_pool(name="apool", bufs=1))
                ptb_pool = p2.enter_context(tc.tile_pool(name="ptb", bufs=3))
                pt8_pool = p2.enter_context(tc.tile_pool(name="pt8", bufs=3))
                spool = p2.enter_context(tc.tile_pool(name="spool", bufs=1))
                ostg = p2.enter_context(tc.tile_pool(name="ostg", bufs=3))
                psS = p2.enter_context(tc.tile_pool(name="psS", bufs=3, space="PSUM"))
                psPO = p2.enter_context(tc.tile_pool(name="psPO", bufs=2, space="PSUM"))
                psR = p2.enter_context(tc.tile_pool(name="psR", bufs=1, space="PSUM"))
                psC = p2.enter_context(tc.tile_pool(name="psC", bufs=2, space="PSUM"))
                vpool = p2.enter_context(tc.tile_pool(name="vpool", bufs=1))

                vbf = vpool.tile([P, SPLIT // P, F], bf16, name="vbf")      # 0.5 MiB
                v8 = vpool.tile([P, L // P // 2, 2, F], f8e4, name="v8")    # 1 MiB
                wob_res = wo_pool.tile([P, H, D], bf16, name="wob_res")
                wo8_res = wo_pool.tile([P, 2, 2, D], f8e4, name="wo8_res")
                wo_loaded = [False]

                def emit_v(jt0):
                    # four k-subtiles' v-projections per q-block boundary
                    for q4 in range(jt0, jt0 + TB // P):
                        if jt0 == 0:
                            emit_v_quarter(q4)
                        else:
                            oproj_queue.append(("v", q4))

                def emit_v_quarter(q4):
                    lo = q4 * P
                    ps = psC.tile([P, F], f32, tag="psC")
                    if q4 < PS // P:
                        for kt in range(KT):
                            nc.tensor.matmul(
                                ps[:],
                                xb_sb[:, kt, lo : lo + P],
                                wvb_res[:, kt],
                                start=(kt == 0),
                                stop=(kt == KT - 1),
                            )
                        sc = 1.0
                    else:
                        llo = lo - PS
                        for ktp in range(KTP):
                            nc.tensor.matmul(
                                ps[:],
                                x8_sb[:, ktp, :, llo : llo + P],
                                wv8_res[:, ktp],
                                start=(ktp == 0),
                                stop=(ktp == KTP - 1),
                                perf_mode=DR,
                            )
                        sc = 1.0 / WS
                    if q4 < SPLIT // P:
                        # bf16 copy for the tb=0 PV stationary
                        nc.scalar.activation(
                            vbf[:, q4], ps[:],
                            mybir.ActivationFunctionType.Identity, scale=sc,
                        )
                    nc.scalar.activation(
                        v8[:, q4 // 2, q4 % 2], ps[:],
                        mybir.ActivationFunctionType.Identity, scale=sc,
                    )

                pts = {}
                po_h = {}
                rsum_h = {}
                att_tb = {}

                def col_off(tb, jt):
                    # fully masked columns left of the diagonal subtiles are
                    # skipped; for tb>0 the offset is shared by the DR pair
                    jl = jt - 4 * tb
                    if jl < 0:
                        return 0
                    if tb == 0:
                        if jl in (1, 2):
                            return jl * P
                        if jl == 3:
                            return 2 * P
                        return 0
                    return 2 * P if jl >= 2 else 0

                def emit_score(tb, h, jt):
                    off = col_off(tb, jt)
                    w = TB - off
                    s = psS.tile([P, TB], f32, tag="s")
                    nc.tensor.matmul(
                        s[:, :w],
                        kT_sb[:, h, jt * P : (jt + 1) * P],
                        qT_sb[:, h, tb * TB + off : (tb + 1) * TB],
                        start=True,
                        stop=True,
                    )
                    jl = jt - 4 * tb
                    if jl >= 0:
                        d0 = jl * P - off
                        if d0 > 0:
                            nc.vector.tensor_scalar_add(s[:, :d0], s[:, :d0], -1e30)
                        nc.vector.tensor_tensor(
                            s[:, d0 : d0 + P],
                            s[:, d0 : d0 + P],
                            tri[:],
                            mybir.AluOpType.add,
                        )
                    if tb == 0:
                        pt = ptb_pool.tile([P, TB], bf16, tag="pt")
                        nc.scalar.activation(
                            pt[:, :w], s[:, :w], mybir.ActivationFunctionType.Exp
                        )
                        pts[(tb, h, jt)] = pt
                    else:
                        if jt % 2 == 0:
                            pts[(tb, h, jt // 2)] = pt8_pool.tile(
                                [P, 2, TB], f8e4, tag="pt8", name=f"pt8_{tb}_{h}_{jt}"
                            )
                        pt8 = pts[(tb, h, jt // 2)]
                        nc.scalar.activation(
                            pt8[:, jt % 2, off:],
                            s[:, :w],
                            mybir.ActivationFunctionType.Exp,
                            bias=m3[:],
                            scale=1.0,
                        )

                def emit_rp(tb, h, jt):
                    njt = 4 * (tb + 1)
                    if tb == 0:
                        off = col_off(tb, jt)
                        w = TB - off
                        pt = pts.pop((tb, h, jt))
                        if jt == 0:
                            po_h[(tb, h)] = psPO.tile([P, TB], f32, tag="po", name=f"po{tb}_{h}")
                            rsum_h[(tb, h)] = psR.tile([16, TB], f32, tag="rsum", name=f"rs{tb}_{h}")
                        nc.tensor.matmul(
                            rsum_h[(tb, h)][0:1, off:], ones_col[:], pt[:, :w],
                            start=(jt == 0), stop=(jt == njt - 1),
                        )
                        nc.tensor.matmul(
                            po_h[(tb, h)][:, off:], vbf[:, jt, h * HD : (h + 1) * HD], pt[:, :w],
                            start=(jt == 0), stop=(jt == njt - 1),
                        )
                        if jt == njt - 1:
                            emit_tail(tb, h)
                        return
                    if jt % 2 == 0:
                        return  # pair completes on odd jt
                    jp = jt // 2
                    njp = njt // 2
                    off = col_off(tb, jt)
                    w = TB - off
                    pt8 = pts.pop((tb, h, jp))
                    if jp == 0:
                        po_h[(tb, h)] = psPO.tile([P, TB], f32, tag="po", name=f"po{tb}_{h}")
                        rsum_h[(tb, h)] = psR.tile([16, TB], f32, tag="rsum", name=f"rs{tb}_{h}")
                    nc.tensor.matmul(
                        rsum_h[(tb, h)][:, off:], ones8[:], pt8[:, :, off:],
                        start=(jp == 0), stop=(jp == njp - 1),
                        perf_mode=DR,
                    )
                    nc.tensor.matmul(
                        po_h[(tb, h)][:, off:], v8[:, jp, :, h * HD : (h + 1) * HD],
                        pt8[:, :, off:],
                        start=(jp == 0), stop=(jp == njp - 1),
                        perf_mode=DR,
                    )
                    if jp == njp - 1:
                        emit_tail(tb, h)

                def emit_tail(tb, h):
                    po = po_h.pop((tb, h))
                    rsum = rsum_h.pop((tb, h))
                    recip = spool.tile([1, TB], f32r, tag="recip")
                    nc.vector.reciprocal(recip[:], rsum[0:1, :])
                    bc_ps = psS.tile([P, TB], f32, tag="s")
                    nc.tensor.matmul(
                        bc_ps[:], ones_row[:], recip[:], start=True, stop=True
                    )
                    bc = spool.tile([P, TB], f32, tag="bc")
                    nc.vector.tensor_scalar_mul(
                        bc[:], bc_ps[:], (1.0 if tb == 0 else ASC)
                    )
                    if tb == 0:
                        nc.vector.tensor_tensor(
                            att_tb[tb][:, h, :], po[:], bc[:], mybir.AluOpType.mult
                        )
                    else:
                        atmp = spool.tile([P, TB], bf16, tag="atmp", bufs=2)
                        nc.vector.tensor_tensor(
                            atmp[:], po[:], bc[:], mybir.AluOpType.mult
                        )
                        nc.scalar.activation(
                            att_tb[tb][:, h // 2, h % 2, :], atmp[:],
                            mybir.ActivationFunctionType.Identity,
                        )

                oproj_queue = []

                def emit_oproj_group(tb, att_sb, tt):
                    # all 4 D-blocks of one 128-row stripe -> one wide
                    # staging tile -> one contiguous 4KB-row DMA
                    ot = ostg.tile([P, D], bf16, tag="ostg")
                    for ob in range(D // TB):
                        ps = psC.tile([P, TB], f32, tag="psC")
                        if tb == 0:
                            for h in range(H):
                                nc.tensor.matmul(
                                    ps[:],
                                    att_sb[:, h, tt * P : (tt + 1) * P],
                                    wob_res[:, h, ob * TB : (ob + 1) * TB],
                                    start=(h == 0),
                                    stop=(h == H - 1),
                                )
                            nc.vector.tensor_scalar_mul(
                                ot[:, ob * TB : (ob + 1) * TB], ps[:], 1.0
                            )
                        else:
                            for hp in range(2):
                                nc.tensor.matmul(
                                    ps[:],
                                    att_sb[:, hp, :, tt * P : (tt + 1) * P],
                                    wo8_res[:, hp, :, ob * TB : (ob + 1) * TB],
                                    start=(hp == 0),
                                    stop=(hp == 1),
                                    perf_mode=DR,
                                )
                            nc.vector.tensor_scalar_mul(
                                ot[:, ob * TB : (ob + 1) * TB], ps[:], 1.0 / (WS * ASC)
                            )
                    nc.sync.dma_start(
                        o[tb * TB + tt * P : tb * TB + (tt + 1) * P, :], ot[:]
                    )

                def emit_oproj(tb):
                    att_sb = att_tb.pop(tb)
                    for tt in range(TB // P):
                        oproj_queue.append(("o", tb, att_sb, tt))

                tasks = []
                for tb in range(NTB):
                    for h in range(H):
                        for jt in range(4 * (tb + 1)):
                            tasks.append((tb, h, jt))

                att_tb[0] = apool.tile([P, H, TB], bf16, tag="attb", name="att0")
                for tb in range(1, NTB):
                    att_tb[tb] = apool.tile(
                        [P, 2, 2, TB], f8e4, tag="att8", bufs=2, name=f"att{tb}"
                    )

                emit_v(0)
                emit_score(*tasks[0])
                emit_score(*tasks[1])
                for i in range(2, len(tasks)):
                    emit_score(*tasks[i])
                    j = i - 2
                    emit_rp(*tasks[j])
                    if i == 8 and not wo_loaded[0]:
                        nc.sync.dma_start(wob_res[:], wob_d)
                        nc.sync.dma_start(wo8_res[:], wo8_d)
                        wo_loaded[0] = True
                    if tasks[j][0] != tasks[j + 1][0]:
                        # j was the last task of its block
                        emit_v((tasks[j][0] + 1) * (TB // P))
                        emit_oproj(tasks[j][0])
                    if oproj_queue:
                        item = oproj_queue.pop(0)
                        if item[0] == "v":
                            emit_v_quarter(item[1])
                        else:
                            emit_oproj_group(*item[1:])
                emit_rp(*tasks[-2])
                emit_rp(*tasks[-1])
                emit_oproj(NTB - 1)
                while oproj_queue:
                    item = oproj_queue.pop(0)
                    if item[0] == "v":
                        emit_v_quarter(item[1])
                    else:
                        emit_oproj_group(*item[1:])

            rep_ctx.close()

    nc.compile()
    _CACHE[key] = nc
    return nc


def _in_maps(hidden_states, Wq, bq, Wk, bk, Wv, bv, Wo, bo):
    hs = np.asarray(hidden_states, np.float32)
    Wq = np.asarray(Wq, np.float32)
    Wk = np.asarray(Wk, np.float32)
    Wv = np.asarray(Wv, np.float32)
    Wo = np.asarray(Wo, np.float32)
    bq = np.asarray(bq, np.float32)
    bk = np.asarray(bk, np.float32)

    e4 = ml_dtypes.float8_e4m3
    b16 = ml_dtypes.bfloat16
    maps = []
    for b in range(B):
        xT = np.ascontiguousarray(hs[b].T)  # [D, L]
        xb = np.ascontiguousarray(
            xT[:, :PS].reshape(KT, P, PS).transpose(1, 0, 2)
        ).astype(b16)
        x8 = np.ascontiguousarray(
            xT[:, PS:].reshape(KTP, 2, P, LT).transpose(2, 0, 1, 3)
        ).astype(e4)
        for g in range(G):
            sl = slice(g * F, (g + 1) * F)
            wqT = Wq[sl, :].T                                 # (D, F)
            wkT = Wk[sl, :].T
            wvT = Wv[sl, :].T
            woT = Wo[:, sl].T                                 # (F, D)
            maps.append(
                {
                    "xb": xb,
                    "x8": x8,
                    "wqb": np.ascontiguousarray(
                        (wqT * SCALE).reshape(KT, P, H, HD).transpose(1, 2, 0, 3)
                    ).astype(b16),
                    "wq8": np.ascontiguousarray(
                        (wqT * WS).reshape(KTP, 2, P, H, HD).transpose(2, 3, 0, 1, 4)
                    ).astype(e4),
                    "wkb": np.ascontiguousarray(
                        wkT.reshape(KT, P, H, HD).transpose(1, 2, 0, 3)
                    ).astype(b16),
                    "wk8": np.ascontiguousarray(
                        (wkT * WS).reshape(KTP, 2, P, H, HD).transpose(2, 3, 0, 1, 4)
                    ).astype(e4),
                    "wvb": np.ascontiguousarray(
                        wvT.reshape(KT, P, F).transpose(1, 0, 2)
                    ).astype(b16),
                    "wv8": np.ascontiguousarray(
                        (wvT * WS).reshape(KTP, 2, P, F).transpose(2, 0, 1, 3)
                    ).astype(e4),
                    "wob": np.ascontiguousarray(
                        woT.reshape(H, HD, D).transpose(1, 0, 2)
                    ).astype(b16),
                    "wo8": np.ascontiguousarray(
                        (woT * WS).reshape(2, 2, HD, D).transpose(2, 0, 1, 3)
                    ).astype(e4),
                    "bq_pp": np.ascontiguousarray((bq[sl] * SCALE).reshape(H, P).T),
                    "bk_pp": np.ascontiguousarray(bk[sl].reshape(H, P).T),
                    "ones_bf": np.ones((P,), b16),
                    "ones8": np.ones((P, 2, 16), e4),
                    "ones_r": np.ones((P,), np.float32),
                }
            )
    return maps


def kernel(hidden_states, Wq, bq, Wk, bk, Wv, bv, Wo, bo, **run_kwargs):
    nc = _build()
    maps = _in_maps(hidden_states, Wq, bq, Wk, bk, Wv, bv, Wo, bo)
    res = bass_utils.run_bass_kernel_spmd(
        nc, maps, core_ids=list(range(8)), **run_kwargs
    )
    bo = np.asarray(bo, np.float32)
    bv = np.asarray(bv, np.float32)
    Wo_f = np.asarray(Wo, np.float32)
    bo_eff = bo + Wo_f @ bv  # softmax weights sum to 1 -> v-bias folds into bo
    out = np.empty((B, L, D), np.float32)
    for b in range(B):
        acc = res.results[b * G]["o"].astype(np.float32).copy()
        for g in range(1, G):
            acc += res.results[b * G + g]["o"]
        out[b] = acc + bo_eff[None, :]
    _CACHE["last_res"] = res
    return out
